# revision 1
# baseline (speedup 1.0000x reference)
"""Masked multi-head attention on 8 Trainium2 NeuronCores.

Sharding: batch x head-group. Core c handles batch c//4 and heads
4*(c%4) .. 4*(c%4)+3 (Wq/Wk/Wv column-sharded, Wo row-sharded). Each core
computes a partial [S, D_MODEL] output = attn_heads @ Wo_slice; the host sums
the 4 partials per batch (the row-parallel reduce) and adds bo + bv @ Wo
(the bv term folds out because softmax rows sum to 1).

Device kernel (per core), all matmuls in float32r (full PE rate, ~1e-4 rel):
  phase-interleaved per 512-wide s block j:
    proj(j): qT/kT [dout, s] via Wq-stationary matmuls, v natural via
             XT-stationary matmuls; then attention for all 4 heads on block j
             (scores transposed [sk, sq], exp without max-subtraction, causal
             triangle mask only on diagonal tiles with fully-masked columns
             skipped, row sums via a fused ones-column in the V stationary);
    then the output projection for the 4 sq tiles of block j.
"""

import numpy as np

D_MODEL = 1024
N_HEAD = 16
HEAD_DIM = 64
B, S = 2, 2048
GH = 4  # heads per core
GC = GH * HEAD_DIM  # 256 dout columns per core
SBK = 512  # s block (moving free dim)
NSB = S // SBK  # 4 s blocks
NKT = D_MODEL // 128  # 8 din tiles
NST = S // 128  # 16 sk tiles

_CACHE = {}


def _build_nc():
    import concourse.mybir as mybir
    from concourse import bacc, tile

    F32 = mybir.dt.float32
    F32R = mybir.dt.float32r
    EXP = mybir.ActivationFunctionType.Exp

    nc = bacc.Bacc(None, target_bir_lowering=False)

    xq = nc.declare_dram_parameter("xq", [D_MODEL, S], F32R, isOutput=False)
    xk = nc.declare_dram_parameter("xk", [D_MODEL, S], F32R, isOutput=False)
    xv = nc.declare_dram_parameter("xv", [D_MODEL, S], F32R, isOutput=False)
    wq = nc.declare_dram_parameter("wq", [D_MODEL, GC], F32R, isOutput=False)
    wk = nc.declare_dram_parameter("wk", [D_MODEL, GC], F32R, isOutput=False)
    wv = nc.declare_dram_parameter("wv", [D_MODEL, GC], F32R, isOutput=False)
    wo = nc.declare_dram_parameter("wo", [GC, D_MODEL], F32R, isOutput=False)
    bq = nc.declare_dram_parameter("bq", [GC, 1], F32, isOutput=False)
    bk = nc.declare_dram_parameter("bk", [GC, 1], F32, isOutput=False)
    y = nc.declare_dram_parameter("y", [S, D_MODEL], F32, isOutput=True)

    with tile.TileContext(nc) as tc:
        with (
            tc.tile_pool(name="res", bufs=1) as res,
            tc.tile_pool(name="work", bufs=3) as work,
            tc.tile_pool(name="xin", bufs=2) as xin,
            tc.tile_pool(name="ps", bufs=2, space="PSUM") as ps,
        ):
            srcs = {"xq": xq, "xk": xk, "xv": xv}

            def load_one(nm, j):
                src = srcs[nm]
                ts = [
                    xin.tile([128, SBK], F32R, tag=f"{nm}{kt % 4}", name=f"{nm}_t_{kt}")
                    for kt in range(NKT)
                ]
                for kt in range(NKT):
                    nc.sync.dma_start(
                        ts[kt][:],
                        src[kt * 128 : (kt + 1) * 128, j * SBK : (j + 1) * SBK],
                    )
                return ts

            # ---- resident weights/biases (interleaved with j=0 activations
            # so the first projection matmuls can start almost immediately) ----
            wq_sb = res.tile([128, NKT, GC], F32R, tag="wq")
            wk_sb = res.tile([128, NKT, GC], F32R, tag="wk")
            wv_sb = res.tile([128, NKT, GC], F32R, tag="wv")
            xq_t = {}
            xk_t = {}
            xv_t = {}
            # xq j0 + Wq interleaved first, then xk j0 + Wk, xv j0 + Wv;
            # prefetch xq j1 between so the q path runs two blocks ahead.
            ts = [
                xin.tile([128, SBK], F32R, tag=f"xq{kt % 4}", name=f"xq_t_{kt}")
                for kt in range(NKT)
            ]
            for kt in range(NKT):
                nc.sync.dma_start(ts[kt][:], xq[kt * 128 : (kt + 1) * 128, 0:SBK])
                nc.sync.dma_start(wq_sb[:, kt], wq[kt * 128 : (kt + 1) * 128, :])
            xq_t[0] = ts
            ts = [
                xin.tile([128, SBK], F32R, tag=f"xk{kt % 4}", name=f"xk_t_{kt}")
                for kt in range(NKT)
            ]
            for kt in range(NKT):
                nc.sync.dma_start(ts[kt][:], xk[kt * 128 : (kt + 1) * 128, 0:SBK])
                nc.sync.dma_start(wk_sb[:, kt], wk[kt * 128 : (kt + 1) * 128, :])
            xk_t[0] = ts
            ts = [
                xin.tile([128, SBK], F32R, tag=f"xv{kt % 4}", name=f"xv_t_{kt}")
                for kt in range(NKT)
            ]
            for kt in range(NKT):
                nc.sync.dma_start(ts[kt][:], xv[kt * 128 : (kt + 1) * 128, 0:SBK])
                nc.sync.dma_start(wv_sb[:, kt], wv[kt * 128 : (kt + 1) * 128, :])
            xv_t[0] = ts
            bq_sb = res.tile([128, 2], F32, tag="bq")
            bk_sb = res.tile([128, 2], F32, tag="bk")
            for pt in range(2):
                nc.sync.dma_start(bq_sb[:, pt : pt + 1], bq[pt * 128 : (pt + 1) * 128, :])
                nc.sync.dma_start(bk_sb[:, pt : pt + 1], bk[pt * 128 : (pt + 1) * 128, :])
            wo_sb = res.tile([128, 2, D_MODEL], F32R, tag="wo")
            for pt in range(2):
                nc.sync.dma_start(wo_sb[:, pt], wo[pt * 128 : (pt + 1) * 128, :])

            # ---- causal triangle mask [128, 128]: keep y >= x ----
            maskt = res.tile([128, 128], F32, tag="maskt")
            nc.gpsimd.memset(maskt[:], 1.0)
            nc.gpsimd.affine_select(
                out=maskt[:],
                in_=maskt[:],
                compare_op=mybir.AluOpType.is_ge,
                fill=0.0,
                base=0,
                pattern=[[1, 128]],
                channel_multiplier=-1,
            )

            # ---- resident activations ----
            qT_sb = [[res.tile([128, SBK], F32R, tag=f"qT_{pt}_{j}", name=f"qT_{pt}_{j}") for j in range(NSB)] for pt in range(2)]
            kT_sb = [[res.tile([128, SBK], F32R, tag=f"kT_{pt}_{j}", name=f"kT_{pt}_{j}") for j in range(NSB)] for pt in range(2)]
            oT_sb = [[res.tile([128, SBK], F32R, tag=f"oT_{pt}_{j}", name=f"oT_{pt}_{j}") for j in range(NSB)] for pt in range(2)]
            # v_aug[jb]: [128, 4(i in block), GH, 65]; cols 0..63 = v, col 64 = 1
            v_aug = [res.tile([128, 4, GH, HEAD_DIM + 1], F32R, tag=f"vaug_{jb}", name=f"vaug_{jb}") for jb in range(NSB)]
            ones_tmp = res.tile([128, 4, GH], F32, tag="ones_tmp")
            nc.vector.memset(ones_tmp[:], 1.0)
            for jb in range(NSB):
                nc.vector.tensor_copy(v_aug[jb][:, :, :, HEAD_DIM], ones_tmp[:])

            for j in range(NSB):
                if j > 0:
                    xq_t[j] = load_one("xq", j)
                    xk_t[j] = load_one("xk", j)
                    xv_t[j] = load_one("xv", j)
                # ---- projections for block j ----
                for pt in range(2):
                    pq = ps.tile([128, SBK], mybir.dt.float32, tag="proj")
                    for kt in range(NKT):
                        nc.tensor.matmul(
                            pq[:],
                            wq_sb[:, kt, pt * 128 : (pt + 1) * 128],
                            xq_t[j][kt][:],
                            start=(kt == 0),
                            stop=(kt == NKT - 1),
                        )
                    nc.vector.tensor_scalar_add(qT_sb[pt][j][:], pq[:], bq_sb[:, pt : pt + 1])
                for pt in range(2):
                    pk = ps.tile([128, SBK], mybir.dt.float32, tag="proj")
                    for kt in range(NKT):
                        nc.tensor.matmul(
                            pk[:],
                            wk_sb[:, kt, pt * 128 : (pt + 1) * 128],
                            xk_t[j][kt][:],
                            start=(kt == 0),
                            stop=(kt == NKT - 1),
                        )
                    nc.vector.tensor_scalar_add(kT_sb[pt][j][:], pk[:], bk_sb[:, pt : pt + 1])
                for st in range(4):
                    pv = ps.tile([128, SBK], mybir.dt.float32, tag="proj")
                    pvs = pv[:, :GC]
                    for kt in range(NKT):
                        nc.tensor.matmul(
                            pvs,
                            xv_t[j][kt][:, st * 128 : (st + 1) * 128],
                            wv_sb[:, kt],
                            start=(kt == 0),
                            stop=(kt == NKT - 1),
                        )
                    pv3 = pvs.rearrange("p (h d) -> p h d", h=GH)
                    nc.vector.tensor_copy(v_aug[j][:, st, :, 0:HEAD_DIM], pv3[:])

                # ---- attention for block j, all heads ----
                n_i = 4 * (j + 1)
                for h in range(GH):
                    pt, po = h // 2, 64 * (h % 2)
                    av = ps.tile([128, SBK], mybir.dt.float32, tag="av")
                    for i in range(n_i):
                        m = i - 4 * j  # >= 0 on diagonal-straddling tiles
                        c0 = 128 * m if m > 0 else 0
                        sc = ps.tile([128, SBK], mybir.dt.float32, tag="scores", bufs=3)
                        nc.tensor.matmul(
                            sc[:, c0:],
                            kT_sb[pt][i // 4][po : po + 64, (i % 4) * 128 : (i % 4 + 1) * 128],
                            qT_sb[pt][j][po : po + 64, c0:],
                            start=True,
                            stop=True,
                        )
                        et = work.tile([128, SBK], F32R, tag="expt", bufs=8)
                        nc.scalar.activation(et[:, c0:], sc[:, c0:], EXP, scale=0.125)
                        if m >= 0:
                            nc.vector.tensor_mul(
                                et[:, c0 : c0 + 128], et[:, c0 : c0 + 128], maskt[:]
                            )
                        nc.tensor.matmul(
                            av[0:65, c0:],
                            v_aug[i // 4][:, i % 4, h, :],
                            et[:, c0:],
                            start=(i == 0),
                            stop=(i == n_i - 1),
                        )
                    with tc.high_priority(offset=64):
                        r_inv = work.tile([128, SBK], F32, tag="r_inv", bufs=2)
                        nc.vector.reciprocal(r_inv[0:1, :], av[64:65, :])
                        rb = work.tile([128, SBK], F32, tag="rb", bufs=2)
                        nc.gpsimd.partition_broadcast(rb[:], r_inv[0:1, :])
                        nc.vector.tensor_mul(
                            oT_sb[pt][j][po : po + 64, :], av[0:64, :], rb[0:64, :]
                        )

                # ---- output projection for the 4 sq tiles of block j ----
                for tt in range(4):
                    c = tt * 128
                    for eb in range(2):
                        yp = ps.tile([128, SBK], mybir.dt.float32, tag="yp", bufs=1)
                        for pt in range(2):
                            nc.tensor.matmul(
                                yp[:],
                                oT_sb[pt][j][:, c : c + 128],
                                wo_sb[:, pt, eb * SBK : (eb + 1) * SBK],
                                start=(pt == 0),
                                stop=(pt == 1),
                            )
                        y_sb = work.tile([128, SBK], F32, tag="y_sb", bufs=4)
                        nc.vector.tensor_copy(y_sb[:], yp[:])
                        t = j * 4 + tt
                        nc.sync.dma_start(
                            y[t * 128 : (t + 1) * 128, eb * SBK : (eb + 1) * SBK],
                            y_sb[:],
                        )
    nc.finalize()
    return nc


def _run_device(Q, K, V, Wq, bq, Wk, bk, Wv, Wo):
    from concourse.bass_utils import run_bass_kernel_spmd

    if "nc" not in _CACHE:
        _CACHE["nc"] = _build_nc()
    nc = _CACHE["nc"]

    in_maps = []
    xT = {}
    for b in range(B):
        xT[("q", b)] = np.ascontiguousarray(Q[b].T)
        xT[("k", b)] = np.ascontiguousarray(K[b].T)
        xT[("v", b)] = np.ascontiguousarray(V[b].T)
    for c in range(8):
        b, g = c // 4, c % 4
        cs = slice(g * GC, (g + 1) * GC)
        in_maps.append(
            {
                "xq": xT[("q", b)],
                "xk": xT[("k", b)],
                "xv": xT[("v", b)],
                "wq": np.ascontiguousarray(Wq[:, cs]),
                "wk": np.ascontiguousarray(Wk[:, cs]),
                "wv": np.ascontiguousarray(Wv[:, cs]),
                "wo": np.ascontiguousarray(Wo[cs, :]),
                "bq": np.ascontiguousarray(bq[cs, None]),
                "bk": np.ascontiguousarray(bk[cs, None]),
            }
        )
    res = run_bass_kernel_spmd(nc, in_maps, core_ids=list(range(8)))
    return res


def kernel(Q, K, V, mask, Wq, bq, Wk, bk, Wv, bv, Wo, bo):
    Q = np.asarray(Q, dtype=np.float32)
    K = np.asarray(K, dtype=np.float32)
    V = np.asarray(V, dtype=np.float32)
    mask = np.asarray(mask)
    Wq, Wk, Wv, Wo = (np.asarray(a, dtype=np.float32) for a in (Wq, Wk, Wv, Wo))
    bq, bk, bv, bo = (np.asarray(a, dtype=np.float32) for a in (bq, bk, bv, bo))

    causal = bool(
        np.array_equal(mask[0], np.tril(np.ones((S, S), dtype=mask.dtype)))
    )
    if not causal:
        return _numpy_fallback(Q, K, V, mask, Wq, bq, Wk, bk, Wv, bv, Wo, bo)

    res = _run_device(Q, K, V, Wq, bq, Wk, bk, Wv, Wo)
    bo_eff = bo + bv @ Wo
    out = np.empty((B, S, D_MODEL), dtype=np.float32)
    for b in range(B):
        acc = res.results[4 * b]["y"].astype(np.float32).copy()
        for g in range(1, 4):
            acc += res.results[4 * b + g]["y"]
        out[b] = acc + bo_eff
    return out


def _numpy_fallback(Q, K, V, mask, Wq, bq, Wk, bk, Wv, bv, Wo, bo):
    out = np.empty((B, S, D_MODEL), dtype=np.float32)
    for b in range(B):
        q = (Q[b] @ Wq + bq).reshape(S, N_HEAD, HEAD_DIM).transpose(1, 0, 2)
        k = (K[b] @ Wk + bk).reshape(S, N_HEAD, HEAD_DIM).transpose(1, 0, 2)
        v = (V[b] @ Wv + bv).reshape(S, N_HEAD, HEAD_DIM).transpose(1, 0, 2)
        mb = mask[b] if mask.shape[0] > 1 else mask[0]
        o = np.empty((N_HEAD, S, HEAD_DIM), dtype=np.float32)
        for hh in range(N_HEAD):
            s = (q[hh] @ k[hh].T) / np.sqrt(np.float32(HEAD_DIM))
            s = np.where(mb == 0, -np.inf, s)
            s = s - s.max(-1, keepdims=True)
            e = np.exp(s)
            p = e / e.sum(-1, keepdims=True)
            o[hh] = p @ v[hh]
        out[b] = o.transpose(1, 0, 2).reshape(S, D_MODEL) @ Wo + bo
    return out

